# revision 2
# baseline (speedup 1.0000x reference)
"""AnyprecisionLinear (w_bits=4) on 8 TRN2 NeuronCores — fp8 DoubleRow version.

kernel(x, qweight, lut) -> out
  x       (1, 2048, 8192) f32
  qweight (8192, 2048)    int32   (4x 8-bit codes per word; idx = code >> 4)
  lut     (8192, 16)      f32
  out     (1, 2048, 8192) f32     == einsum('bsk,ok->bso', x, lut[o, idx[o,k]])

Column-parallel: core i owns output rows [1024*i, 1024*(i+1)).

Numerics: W = wh + wl (two e4m3 planes from the LUT), x = x8 + xl (host
split).  out ~= wh@x8 + wl@x8 + wh@xl (xl@wl term ~5e-4, dropped); all three
terms run as fp8 DoubleRow matmuls (2 k-subtiles per instruction, 0.5
cyc/row).

Per-core pipeline (2 o-groups of 4 o-tiles):
  - LUT entries are u16 (wh|wl<<8); 8 table words T_m (u32 = entry pair) per
    o-row; host ships tb_t = T_2t, dl_t = T_2t^T_2t+1.
  - Dequant per (128,1024) code chunk, chunk-major across the group's 4
    o-tiles so the first matmuls can start after one chunk-column: Pool
    computes the byte masks (t1/m2/b0/m1/m3), DVE does the 4 masked-xor
    table ops + 4 predicated-copy selects, Act densifies the strided u16
    result, and the DMA xbar transposes (o,k) -> [128 k, 8 blk, 128 o] u16
    straight into the DoubleRow stationary layout.  No PE transposes.
  - x is shipped host-packed as u16 (x8|xl<<8) (S,K); Act-issued DMA
    transposes per (512 s, 256 k) slab into [128, 2, 512] u16 = both fp8
    planes of a k-subtile pair; matmuls read byte-strided fp8 views.
  - PSUM: 4 accumulators/group double-buffered over 4 s-blocks = 8 banks.
"""
import numpy as np
import ml_dtypes

import concourse.mybir as mybir
from concourse import bacc, bass_utils
from concourse.tile import TileContext

dt = mybir.dt
A = mybir.AluOpType
F8 = ml_dtypes.float8_e4m3

O, K, S = 1024, 8192, 2048     # per-core out rows, contraction, tokens
P = 128
OT = O // P                    # 8 o-tiles
KC = 1024                      # dequant k-chunk
NCH = K // KC                  # 8 chunks per o-tile
NBPC = KC // 256               # 4 block-pairs per chunk
NBP = K // 256                 # 32 block-pairs
SBW = 512
NSB = S // SBW                 # 4 s-blocks
GROUPS = [(0, 4), (4, 4)]
N_CORES = 8
DR = mybir.MatmulPerfMode.DoubleRow


def _host_tables(lut_shard):
    wh = lut_shard.astype(F8)
    wl = (lut_shard - wh.astype(np.float32)).astype(F8)
    U = wh.view(np.uint8).astype(np.uint32) | (wl.view(np.uint8).astype(np.uint32) << 8)
    T = U[:, 0::2] | (U[:, 1::2] << 16)
    tb = T[:, 0::2].copy()
    dl = (T[:, 0::2] ^ T[:, 1::2]).copy()
    return tb, dl


def _build_kernel(nc):
    xu_in = nc.declare_dram_parameter("xu", [S, K], dt.uint16, isOutput=False)
    qw_in = nc.declare_dram_parameter("qw", [O, K], dt.uint8, isOutput=False)
    tb_in = nc.declare_dram_parameter("tb", [O, 4], dt.uint32, isOutput=False)
    dl_in = nc.declare_dram_parameter("dl", [O, 4], dt.uint32, isOutput=False)
    out_d = nc.declare_dram_parameter("out", [O, S], dt.float32, isOutput=True)

    with TileContext(nc) as tc:
        with tc.tile_pool(name="wt", bufs=1) as wtpool, \
             tc.tile_pool(name="tabs", bufs=1) as tabpool, \
             tc.tile_pool(name="deq", bufs=1) as dqpool, \
             tc.tile_pool(name="xt", bufs=6) as xtpool, \
             tc.tile_pool(name="outd", bufs=3) as outpool, \
             tc.tile_pool(name="psacc", bufs=1, space="PSUM") as psacc:

            tabs = []
            for ot in range(OT):
                tb_sb = tabpool.tile([P, 4], dt.uint32, name=f"tb{ot}")
                dl_sb = tabpool.tile([P, 4], dt.uint32, name=f"dl{ot}")
                nc.sync.dma_start(out=tb_sb, in_=tb_in[ot * P:(ot + 1) * P, :])
                nc.sync.dma_start(out=dl_sb, in_=dl_in[ot * P:(ot + 1) * P, :])
                tabs.append((tb_sb, dl_sb))

            # stationary W: per (o-tile, chunk) a [128 k, 8 blk, 128 o] u16 tile
            wtc = [[wtpool.tile([P, NCH, P], dt.uint16, name=f"wt{ot}_{ch}")
                    for ch in range(NCH)] for ot in range(OT)]

            def dequant_chunk(ot, ch, qslot):
                tb_sb, dl_sb = tabs[ot]
                c8 = dqpool.tile([P, KC], dt.uint8, name="qwc", tag=f"qw{qslot}", bufs=2)
                nc.sync.dma_start(out=c8, in_=qw_in[ot * P:(ot + 1) * P,
                                                    ch * KC:(ch + 1) * KC])
                cw = c8.bitcast(dt.uint32)
                t1 = dqpool.tile([P, KC // 4], dt.uint32, name="t1", tag="t1", bufs=2)
                nc.vector.tensor_scalar(out=t1, in0=cw, scalar1=5, scalar2=0x01010101,
                                        op0=A.logical_shift_right, op1=A.bitwise_and)
                m2 = dqpool.tile([P, KC // 4], dt.uint32, name="m2", tag="m2", bufs=2)
                nc.vector.tensor_scalar(out=m2, in0=cw, scalar1=0x40404040, scalar2=None,
                                        op0=A.bitwise_and)
                m3 = dqpool.tile([P, KC], dt.uint8, name="m3", tag="m3", bufs=2)
                nc.gpsimd.tensor_scalar(out=m3, in0=c8, scalar1=128.0, scalar2=None,
                                        op0=A.is_ge)
                b0 = dqpool.tile([P, KC // 4], dt.uint32, name="b0", tag="b0", bufs=2)
                nc.vector.tensor_scalar(out=b0, in0=cw, scalar1=0x10101010, scalar2=None,
                                        op0=A.bitwise_and)
                m1 = dqpool.tile([P, KC], dt.int32, name="m1", tag="m1", bufs=2)
                nc.gpsimd.tensor_scalar(out=m1, in0=t1.bitcast(dt.uint8), scalar1=-1.0,
                                        scalar2=None, op0=A.mult)

                zs = []
                for t in range(4):
                    z = dqpool.tile([P, KC], dt.uint32, name=f"z{t}", tag=f"z{t}",
                                    bufs=2 if t == 0 else 1)
                    nc.vector.tensor_scalar(out=z, in0=m1.bitcast(dt.uint32),
                                            scalar1=dl_sb[:, t:t + 1], scalar2=tb_sb[:, t:t + 1],
                                            op0=A.bitwise_and, op1=A.bitwise_xor)
                    zs.append(z)
                nc.vector.copy_predicated(out=zs[0], mask=m2.bitcast(dt.uint8), data=zs[1])
                nc.vector.copy_predicated(out=zs[2], mask=m2.bitcast(dt.uint8), data=zs[3])
                nc.vector.copy_predicated(out=zs[0], mask=m3, data=zs[2])

                zv = zs[0].bitcast(dt.uint16).rearrange("p (k two) -> p k two", two=2)
                nc.vector.copy_predicated(out=zv[:, :, 0], mask=b0.bitcast(dt.uint8),
                                          data=zv[:, :, 1])

                zd = dqpool.tile([P, KC], dt.uint16, name="zd", tag="zd", bufs=2)
                nc.scalar.copy(out=zd, in_=zv[:, :, 0])
                nc.sync.dma_start_transpose(out=wtc[ot][ch], in_=zd)

            def matmul_group(g0, gn):
                for half in range(2):
                    sbs = [2 * half, 2 * half + 1]
                    accs = {(i, sb): psacc.tile([P, SBW], dt.float32,
                                                name=f"acc{g0}_{half}_{i}_{sb}",
                                                tag=f"acc{i}_{sb % 2}")
                            for i in range(gn) for sb in sbs}
                    for bp in range(NBP):
                        ch, bpl = divmod(bp, NBPC)
                        for sb in sbs:
                            xt = xtpool.tile([P, 2, SBW], dt.uint16, name="xt", tag="xt")
                            nc.sync.dma_start_transpose(
                                out=xt,
                                in_=xu_in[sb * SBW:(sb + 1) * SBW, bp * 256:(bp + 1) * 256])
                            xt8 = xt.bitcast(dt.float8e4).rearrange(
                                "p a (s two) -> p a s two", two=2)
                            x8ap = xt8[:, :, :, 0]
                            xlap = xt8[:, :, :, 1]
                            for i in range(gn):
                                ot = g0 + i
                                w8 = wtc[ot][ch].bitcast(dt.float8e4).rearrange(
                                    "p a (b two) -> p a b two", two=2)
                                wh = w8[:, 2 * bpl:2 * bpl + 2, :, 0]
                                wl = w8[:, 2 * bpl:2 * bpl + 2, :, 1]
                                acc = accs[(i, sb)]
                                nc.tensor.matmul(acc, wh, x8ap, start=(bp == 0), stop=False,
                                                 perf_mode=DR)
                                nc.tensor.matmul(acc, wl, x8ap, start=False, stop=False,
                                                 perf_mode=DR)
                                nc.tensor.matmul(acc, wh, xlap, start=False,
                                                 stop=(bp == NBP - 1), perf_mode=DR)
                    for (i, sb), acc in accs.items():
                        ot = g0 + i
                        ob = outpool.tile([P, SBW], dt.float32, name="ob", tag="ob")
                        nc.scalar.copy(out=ob, in_=acc)
                        nc.sync.dma_start(
                            out=out_d[ot * P:(ot + 1) * P, sb * SBW:(sb + 1) * SBW], in_=ob)

            for (g0, gn) in GROUPS:
                for ch in range(NCH):
                    for i in range(gn):
                        dequant_chunk(g0 + i, ch, i)
                matmul_group(g0, gn)


_NC_CACHE = None


def _get_nc():
    global _NC_CACHE
    if _NC_CACHE is None:
        nc = bacc.Bacc("TRN2", num_devices=N_CORES)
        _build_kernel(nc)
        nc.compile()
        _NC_CACHE = nc
    return _NC_CACHE


def kernel(x, qweight, lut):
    x = np.asarray(x)
    qweight = np.asarray(qweight)
    lut = np.asarray(lut)
    x2 = np.ascontiguousarray(x.reshape(S, K).astype(np.float32, copy=False))

    x8 = x2.astype(F8)
    xl = (x2 - x8.astype(np.float32)).astype(F8)
    xu = (x8.view(np.uint8).astype(np.uint16)
          | (xl.view(np.uint8).astype(np.uint16) << 8))

    in_maps = []
    for c in range(N_CORES):
        o0, o1 = c * O, (c + 1) * O
        qb = np.ascontiguousarray(qweight[o0:o1]).view(np.uint8).reshape(O, K)
        tb, dl = _host_tables(lut[o0:o1])
        in_maps.append({"xu": xu, "qw": qb, "tb": tb, "dl": dl})

    nc = _get_nc()
    res = bass_utils.run_bass_kernel_spmd(nc, in_maps, core_ids=list(range(N_CORES)))
    out_full = np.concatenate([res.results[c]["out"] for c in range(N_CORES)], axis=0)
    return np.ascontiguousarray(out_full.T).reshape(1, S, 8192).astype(np.float32, copy=False)
